# revision 1
# baseline (speedup 1.0000x reference)
"""Disentangled MHA (DeBERTa-style) Trainium2 Bass kernel.

Sharding: 16 heads across 8 cores (2 heads/core), batch kept local.
Per core: project q/k/v with a 128-column weight slice, build the
relative-position score bands, skew-gather them via a DRAM round trip,
softmax (transposed orientation, unnormalized-exp + fused Z column),
and PV matmul. Host concatenates the per-core 128-feature outputs.

B=4, S=512, DIM=1024, H=16, HD=64, MAX_REL=512.
"""

import numpy as np

import concourse.bass as bass
import concourse.bacc as bacc
import concourse.mybir as mybir
import concourse.tile as tile
from concourse.bass_utils import run_bass_kernel_spmd
from concourse.masks import make_identity

B, S, DIM, H, HD = 4, 512, 1024, 16, 64
T = B * S                      # 2048 tokens
R = 1024                       # 2 * att_span rel rows
HC = 2                         # heads per core
NCORES = 8
KC = DIM // 128                # contraction chunks
SCALE = float((HD * 3) ** (-0.5))
BAND = 640                     # skew band width (needs >= 512 + 127)

F32 = mybir.dt.float32
F32R = mybir.dt.float32r
F16 = mybir.dt.float16
AF = mybir.ActivationFunctionType
ALU = mybir.AluOpType


def _r32(ap):
    return ap.bitcast(F32R)


def build_nc():
    nc = bacc.Bacc("TRN2", target_bir_lowering=False, debug=False)

    xT_d = nc.dram_tensor("xT", [DIM, T], F16, kind="ExternalInput")
    relT_d = nc.dram_tensor("relT", [DIM, R], F16, kind="ExternalInput")
    W_d = {
        n: nc.dram_tensor(f"W{n}", [DIM, 128], F16, kind="ExternalInput")
        for n in "qkv"
    }
    b_d = {
        n: nc.dram_tensor(f"b{n}", [128, 1], F32, kind="ExternalInput")
        for n in "qkv"
    }
    out_d = nc.dram_tensor("out", [T, 128], F32, kind="ExternalOutput")

    with tile.TileContext(nc) as tc:
        _body(nc, tc, xT_d.ap(), relT_d.ap(),
              {n: W_d[n].ap() for n in "qkv"},
              {n: b_d[n].ap() for n in "qkv"},
              out_d.ap())
    nc.compile()
    return nc


def _body(nc, tc, xT, relT, W, bvec, out_d):
    from contextlib import ExitStack
    ctx = ExitStack()
    with ctx:
        singles = ctx.enter_context(tc.tile_pool(name="singles", bufs=1))

        # ---- Load inputs: few big batched DMAs so compute starts early.
        # relT + W first (posk/posq only need these); xT split per batch.
        relT_sb = singles.tile([128, KC * R], F16, name="relT_sb")
        nc.scalar.dma_start(
            out=relT_sb.rearrange("p (i r) -> p i r", i=KC),
            in_=bass.AP(relT.tensor, relT.offset,
                        [[R, 128], [128 * R, KC], [1, R]]))
        relT_t = [relT_sb[:, i * R:(i + 1) * R] for i in range(KC)]

        W_t = {}
        for wi, n in enumerate("qkv"):
            wsb = singles.tile([128, KC * 128], F16, name=f"W{n}_sb")
            nc.sync.dma_start(
                out=wsb.rearrange("p (i c) -> p i c", i=KC),
                in_=bass.AP(W[n].tensor, W[n].offset,
                            [[128, 128], [128 * 128, KC], [1, 128]]))
            W_t[n] = [wsb[:, i * 128:(i + 1) * 128] for i in range(KC)]

        xT_sb = singles.tile([128, KC * T], F16, name="xT_sb")
        xq = [nc.sync, nc.scalar, nc.gpsimd, nc.sync]
        for b in range(B):
            nc_dst = xT_sb.rearrange("p (i t) -> p i t", i=KC)
            nc_dst = nc_dst[:, :, b * S:(b + 1) * S]
            xq[b].dma_start(
                out=nc_dst,
                in_=bass.AP(xT.tensor, xT.offset + b * S,
                            [[T, 128], [128 * T, KC], [1, S]]))
        xT_t = [xT_sb[:, i * T:(i + 1) * T] for i in range(KC)]

        b_t = {}
        for n in "qkv":
            b_t[n] = singles.tile([128, 1], F32, name=f"b{n}")
            nc.gpsimd.dma_start(out=b_t[n], in_=bvec[n])

        ident = singles.tile([128, 128], F16, name="ident")
        make_identity(nc, ident)

        # ---- Phase A: projections (fp32r matmuls, fp16 outputs) ----
        q2T = singles.tile([128, T], F16, name="q2T")
        k2T = singles.tile([128, T], F16, name="k2T")
        v2T = singles.tile([128, T], F16, name="v2T")
        posk = singles.tile([128, R], F16, name="posk")
        posq = singles.tile([128, R], F16, name="posq")

        vtok = []
        for t in range(T // 128):
            vt = singles.tile([128, 130], F16, name=f"vtok{t}")
            vtok.append(vt)

        # rel-dependent projections first (only need relT+W, ~8MB less DMA
        # than xT), then per-batch x projections so batch 0 is ready early
        cast_flip = [0]
        with tc.tile_pool(name="psA", space="PSUM", bufs=2) as psA, \
                tc.tile_pool(name="psVT", space="PSUM", bufs=2) as psVT:

            def proj_tile(out_sb, rhs_tiles, wn, nt):
                ps = psA.tile([128, 512], F32, name="ps_proj", tag="ps_proj")
                for kc in range(KC):
                    nc.tensor.matmul(
                        out=ps,
                        lhsT=W_t[wn][kc][:, :],
                        rhs=rhs_tiles[kc][:, nt * 512:(nt + 1) * 512],
                        start=(kc == 0), stop=(kc == KC - 1),
                    )
                # cast f32->f16 + per-partition bias add
                nc.scalar.activation(
                    out=out_sb[:, nt * 512:(nt + 1) * 512], in_=ps,
                    func=AF.Identity, bias=b_t[wn], scale=1.0,
                )

            for nt in range(R // 512):
                proj_tile(posk, relT_t, "k", nt)
                proj_tile(posq, relT_t, "q", nt)
            for b in range(T // 512):
                proj_tile(q2T, xT_t, "q", b)
                proj_tile(k2T, xT_t, "k", b)
                proj_tile(v2T, xT_t, "v", b)
                # v_tok: transpose batch b of v2T token-major, ones col
                for t in range(b * 4, b * 4 + 4):
                    pst = psVT.tile([128, 128], F16, name="ps_vt", tag="ps_vt")
                    nc.tensor.transpose(pst, v2T[:, t * 128:(t + 1) * 128],
                                        ident)
                    nc.vector.tensor_copy(vtok[t][:, 0:64], pst[:, 0:64])
                    nc.vector.tensor_copy(vtok[t][:, 65:129], pst[:, 64:128])
                    nc.gpsimd.memset(vtok[t][:, 64:65], 1.0)
                    nc.gpsimd.memset(vtok[t][:, 129:130], 1.0)

        # ---- Phase B ----
        band_dram = ctx.enter_context(
            tc.tile_pool(name="bands", space="DRAM", bufs=1))
        sb_band = ctx.enter_context(tc.tile_pool(name="sb_band", bufs=3))
        sb_work = ctx.enter_context(tc.tile_pool(name="sb_work", bufs=4))
        sb_out = ctx.enter_context(tc.tile_pool(name="sb_out", bufs=3))
        ps_band_pool = ctx.enter_context(
            tc.tile_pool(name="psBand", space="PSUM", bufs=2))
        ps_qk_pool = ctx.enter_context(
            tc.tile_pool(name="psQK", space="PSUM", bufs=2))
        ps_pv_pool = ctx.enter_context(
            tc.tile_pool(name="psPV", space="PSUM", bufs=2))

        copy_flip = [0]

        def psum_to_sbuf_f16(dst, src):
            # 2:1 DVE:ACT — Scalar(ACT) is the busiest queue (exp + casts)
            if copy_flip[0] % 3 != 2:
                nc.vector.tensor_copy(dst, src)
            else:
                nc.scalar.copy(dst, src)
            copy_flip[0] += 1

        # --- B1: score bands for one batch: matmul -> sbuf -> dram ---
        c2p_bd = {}         # (b, h) -> dram tile [512, 1024] (pitch-1024 skew)
        p2c_bd = {}         # (b, h) -> dram tile [128, 4*BAND]

        def emit_b1(b):
            csb = {h: sb_band.tile([128, 4 * BAND], F16, name=f"c2p_sb{h}",
                                   tag=f"c2p_sb{h}", bufs=2) for h in range(HC)}
            psb = {h: sb_band.tile([128, 4 * BAND], F16, name=f"p2c_sb{h}",
                                   tag=f"p2c_sb{h}", bufs=2) for h in range(HC)}
            for blk in range(4):
                c0 = 128 * (3 - blk)
                cs = slice(b * 512 + blk * 128, b * 512 + (blk + 1) * 128)
                for src2T, pos, stage in ((q2T, posk, csb), (k2T, posq, psb)):
                    pss = []
                    for h in range(HC):   # adjacent h matmuls -> PE row packing
                        hs = slice(h * 64, (h + 1) * 64)
                        ps = ps_band_pool.tile([128, BAND], F32,
                                               name="ps_band", tag="ps_band")
                        nc.tensor.matmul(
                            out=ps[:, 0:512], lhsT=src2T[hs, cs],
                            rhs=pos[hs, c0:c0 + 512], start=True, stop=True)
                        nc.tensor.matmul(
                            out=ps[:, 512:BAND], lhsT=src2T[hs, cs],
                            rhs=pos[hs, c0 + 512:c0 + BAND],
                            start=True, stop=True)
                        pss.append(ps)
                    for h in range(HC):
                        psum_to_sbuf_f16(
                            stage[h][:, blk * BAND:(blk + 1) * BAND], pss[h])
            for h in range(HC):
                # c2p: strided write into a [512, 1024]-pitch buffer so the
                # transposed skew read is a single 2D AP (offset q*1023+k+512)
                bdr = band_dram.tile([512, 1024], F16, name=f"c2pb_{b}{h}",
                                     tag=f"c2p_dram_{b}{h}", bufs=1)
                dst = bass.AP(bdr.tensor, bdr.offset + 384,
                              [[1024, 128], [130944, 4], [1, BAND]])
                nc.sync.dma_start(
                    out=dst, in_=csb[h].rearrange("p (g j) -> p g j", g=4))
                c2p_bd[(b, h)] = bdr
                # p2c: flat [128, 4*BAND]; skew read offset ki*2559+kb*640+128+q
                bdr = band_dram.tile([128, 4 * BAND], F16, name=f"p2cb_{b}{h}",
                                     tag=f"p2c_dram_{b}{h}", bufs=1)
                nc.sync.dma_start(out=bdr, in_=psb[h])
                p2c_bd[(b, h)] = bdr

        # --- B2: attention for one batch ---
        # constant exp bias keeps f16 E and the f16-transposed Z in range;
        # it cancels exactly in the final E@v / Z normalization
        exp_bias = singles.tile([128, 1], F32, name="exp_bias")
        nc.gpsimd.memset(exp_bias, -4.0)

        def emit_b2(b):
            ostage = sb_out.tile([128, 512], F32, name="ostage", tag="ostage")
            for h in range(HC):
                hs = slice(h * 64, (h + 1) * 64)
                ps_pv = ps_pv_pool.tile([65, 512], F32, name="ps_pv", tag="ps_pv")
                for kb in range(4):
                    ks = slice(b * 512 + kb * 128, b * 512 + (kb + 1) * 128)
                    # qkT: [k 128, q 512]
                    ps_qk = ps_qk_pool.tile([128, 512], F32,
                                            name="ps_qk", tag="ps_qk")
                    nc.tensor.matmul(
                        out=ps_qk, lhsT=k2T[hs, ks],
                        rhs=q2T[hs, b * 512:(b + 1) * 512],
                        start=True, stop=True)

                    # c2pT: one transposed skew read over the full-pitch band
                    t_sb = sb_work.tile([128, 512], F16, name="t_sb",
                                        tag="t_sb", bufs=16)
                    bdr = c2p_bd[(b, h)]
                    src = bass.AP(bdr.tensor, bdr.offset + 512 + 128 * kb,
                                  [[1023, 512], [1, 128]])
                    nc.sync.dma_start_transpose(out=t_sb, in_=src)
                    # p2cT: accumulate plain skew read
                    bdr = p2c_bd[(b, h)]
                    src = bass.AP(bdr.tensor, bdr.offset + kb * BAND + 128,
                                  [[4 * BAND - 1, 128], [1, 512]])
                    nc.gpsimd.dma_start(out=t_sb, in_=src, accum_op=ALU.add)

                    # scores + exp (unnormalized, transposed)
                    s_sb = sb_work.tile([128, 512], F16, name="s_sb",
                                        tag="s_sb", bufs=12)
                    nc.vector.tensor_tensor(out=s_sb, in0=t_sb, in1=ps_qk,
                                            op=ALU.add)
                    eT = sb_work.tile([128, 512], F16, name="eT", tag="eT",
                                      bufs=12)
                    nc.scalar.activation(out=eT, in_=s_sb, func=AF.Exp,
                                         scale=SCALE, bias=exp_bias)
                    # PV with stationary [v|1]: psum [65, 512] = (v|1)^T @ E^T
                    nc.tensor.matmul(
                        out=ps_pv, lhsT=vtok[b * 4 + kb][:, h * 65:h * 65 + 65],
                        rhs=eT, start=(kb == 0), stop=(kb == 3))

                # --- finalize: out^T [65, 512] -> transpose -> /Z -> stage ---
                o2T = sb_work.tile([65, 512], F16, name="o2T", tag="o2T")
                nc.scalar.copy(o2T, ps_pv)
                for qc in range(4):
                    psT = ps_band_pool.tile([128, 65], F16, name="psT",
                                            tag="ps_band")
                    nc.tensor.transpose(psT, o2T[:, qc * 128:(qc + 1) * 128],
                                        ident[0:65, 0:65])
                    zrec = sb_work.tile([128, 1], F32, name="zrec",
                                        tag="zrec", bufs=8)
                    nc.vector.reciprocal(zrec, psT[:, 64:65])
                    nc.vector.tensor_scalar_mul(
                        ostage[:, qc * 128 + h * 64:qc * 128 + (h + 1) * 64],
                        psT[:, 0:64], zrec)
            # one merged output write per batch
            dst = bass.AP(out_d.tensor, out_d.offset + b * 65536,
                          [[128, 128], [16384, 4], [1, 128]])
            nc.gpsimd.dma_start(
                out=dst, in_=ostage.rearrange("p (g j) -> p g j", g=4))

        # software-pipelined emission: bands stay two batches ahead of the
        # attention consuming them, so no queue's FIFO head blocks on work
        # that hasn't been produced yet
        emit_b1(0)
        emit_b1(1)
        emit_b2(0)
        emit_b1(2)
        emit_b2(1)
        emit_b1(3)
        emit_b2(2)
        emit_b2(3)


_NC_CACHE = None


def _get_nc():
    global _NC_CACHE
    if _NC_CACHE is None:
        _NC_CACHE = build_nc()
    return _NC_CACHE


def make_in_maps(inputs):
    x = np.asarray(inputs["x"], np.float32)
    rel = np.asarray(inputs["rel_embeddings"], np.float32)
    Wq = np.asarray(inputs["Wq"], np.float32)
    Wk = np.asarray(inputs["Wk"], np.float32)
    Wv = np.asarray(inputs["Wv"], np.float32)
    bq = np.asarray(inputs["bq"], np.float32)
    bk = np.asarray(inputs["bk"], np.float32)
    bv = np.asarray(inputs["bv"], np.float32)

    xT = np.ascontiguousarray(x.reshape(T, DIM).T).astype(np.float16)
    relT = np.ascontiguousarray(rel[::-1].T).astype(np.float16)
    in_maps = []
    for c in range(NCORES):
        sl = slice(c * 128, (c + 1) * 128)
        in_maps.append({
            "xT": xT,
            "relT": relT,
            "Wq": np.ascontiguousarray(Wq[:, sl]).astype(np.float16),
            "Wk": np.ascontiguousarray(Wk[:, sl]).astype(np.float16),
            "Wv": np.ascontiguousarray(Wv[:, sl]).astype(np.float16),
            "bq": np.ascontiguousarray(bq[sl]).reshape(128, 1),
            "bk": np.ascontiguousarray(bk[sl]).reshape(128, 1),
            "bv": np.ascontiguousarray(bv[sl]).reshape(128, 1),
        })
    return in_maps


def kernel(**inputs):
    nc = _get_nc()
    in_maps = make_in_maps(inputs)
    res = run_bass_kernel_spmd(nc, in_maps, list(range(NCORES))).results
    out = np.concatenate([res[c]["out"] for c in range(NCORES)], axis=1)
    return out.reshape(B, S, DIM).astype(np.float32)



# revision 2
# speedup vs baseline: 1.3219x; 1.3219x over previous
"""Disentangled MHA (DeBERTa-style) Trainium2 Bass kernel.

Sharding: 16 heads across 8 cores (2 heads/core), batch kept local.
Per core: project q/k/v with a 128-column weight slice, build the
relative-position score bands, skew-gather them via a DRAM round trip,
softmax (transposed orientation, unnormalized-exp + fused Z column),
and PV matmul. Host concatenates the per-core 128-feature outputs.

v2: per-batch interleaved emission (proj -> bands -> attention as a
3-deep software pipeline), DMA spread over the three issuing queues
(sync/scalar HWDGE + gpsimd SWDGE), merged skew reads (one transpose
DMA + one accumulating DMA per (batch, head)), single shared PSUM pool
sized to the 8 banks.

B=4, S=512, DIM=1024, H=16, HD=64, MAX_REL=512.
"""

import numpy as np

import concourse.bass as bass
import concourse.bacc as bacc
import concourse.mybir as mybir
import concourse.tile as tile
from concourse.bass_utils import run_bass_kernel_spmd
from concourse.masks import make_identity

B, S, DIM, H, HD = 4, 512, 1024, 16, 64
T = B * S                      # 2048 tokens
R = 1024                       # 2 * att_span rel rows
HC = 2                         # heads per core
NCORES = 8
KC = DIM // 128                # contraction chunks
SCALE = float((HD * 3) ** (-0.5))
BAND = 640                     # skew band width (needs >= 512 + 127)

F32 = mybir.dt.float32
F16 = mybir.dt.float16
AF = mybir.ActivationFunctionType
ALU = mybir.AluOpType


def build_nc():
    nc = bacc.Bacc("TRN2", target_bir_lowering=False, debug=False)

    xT_d = nc.dram_tensor("xT", [DIM, T], F16, kind="ExternalInput")
    relT_d = nc.dram_tensor("relT", [DIM, R], F16, kind="ExternalInput")
    W_d = {
        n: nc.dram_tensor(f"W{n}", [DIM, 128], F16, kind="ExternalInput")
        for n in "qkv"
    }
    b_d = {
        n: nc.dram_tensor(f"b{n}", [128, 1], F32, kind="ExternalInput")
        for n in "qkv"
    }
    out_d = nc.dram_tensor("out", [T, 128], F32, kind="ExternalOutput")

    with tile.TileContext(nc) as tc:
        _body(nc, tc, xT_d.ap(), relT_d.ap(),
              {n: W_d[n].ap() for n in "qkv"},
              {n: b_d[n].ap() for n in "qkv"},
              out_d.ap())
    nc.compile()
    return nc


def _body(nc, tc, xT, relT, W, bvec, out_d):
    from contextlib import ExitStack
    ctx = ExitStack()
    with ctx:
        singles = ctx.enter_context(tc.tile_pool(name="singles", bufs=1))

        # ---- Input loads, spread over the three DMA-issuing queues.
        # sync: relT lo-half, W (posk/posq prereqs), xT b0/b3
        # scalar: relT hi-half, xT b1;  gpsimd: xT b2, biases
        relT_sb = singles.tile([128, KC * R], F16, name="relT_sb")
        relT_r = relT_sb.rearrange("p (i r) -> p i r", i=KC)
        nc.sync.dma_start(
            out=relT_r[:, 0:4],
            in_=bass.AP(relT.tensor, relT.offset,
                        [[R, 128], [128 * R, 4], [1, R]]))
        nc.scalar.dma_start(
            out=relT_r[:, 4:8],
            in_=bass.AP(relT.tensor, relT.offset + 4 * 128 * R,
                        [[R, 128], [128 * R, 4], [1, R]]))
        relT_t = [relT_sb[:, i * R:(i + 1) * R] for i in range(KC)]

        W_t = {}
        for n in "kqv":
            wsb = singles.tile([128, KC * 128], F16, name=f"W{n}_sb")
            nc.sync.dma_start(
                out=wsb.rearrange("p (i c) -> p i c", i=KC),
                in_=bass.AP(W[n].tensor, W[n].offset,
                            [[128, 128], [128 * 128, KC], [1, 128]]))
            W_t[n] = [wsb[:, i * 128:(i + 1) * 128] for i in range(KC)]

        xT_sb = singles.tile([128, KC * T], F16, name="xT_sb")
        xT_r = xT_sb.rearrange("p (i t) -> p i t", i=KC)
        xq = [nc.sync, nc.scalar, nc.gpsimd, nc.sync]
        for b in range(B):
            xq[b].dma_start(
                out=xT_r[:, :, b * S:(b + 1) * S],
                in_=bass.AP(xT.tensor, xT.offset + b * S,
                            [[T, 128], [128 * T, KC], [1, S]]))
        xT_t = [xT_sb[:, i * T:(i + 1) * T] for i in range(KC)]

        b_t = {}
        for n in "qkv":
            b_t[n] = singles.tile([128, 1], F32, name=f"b{n}")
            nc.gpsimd.dma_start(out=b_t[n], in_=bvec[n])

        ident = singles.tile([128, 128], F16, name="ident")
        make_identity(nc, ident)

        exp_bias = singles.tile([128, 1], F32, name="exp_bias")
        nc.gpsimd.memset(exp_bias, -4.0)

        # ---- persistent SBUF for projections ----
        q2T = singles.tile([128, T], F16, name="q2T")
        k2T = singles.tile([128, T], F16, name="k2T")
        v2T = singles.tile([128, T], F16, name="v2T")
        posk = singles.tile([128, R], F16, name="posk")
        posq = singles.tile([128, R], F16, name="posq")
        vtok = [singles.tile([128, 130], F16, name=f"vtok{t}")
                for t in range(T // 128)]

        # ---- single PSUM pool, 8 banks total ----
        # ps512 (proj + band main + band leftover): 3 banks
        # ps_qk: 2, ps_pv: 2, ps_t (f16 transposes): 1
        ps = ctx.enter_context(tc.tile_pool(name="ps", space="PSUM", bufs=1))

        # ---- SBUF pools ----
        sb_band = ctx.enter_context(tc.tile_pool(name="sb_band", bufs=1))
        sb_work = ctx.enter_context(tc.tile_pool(name="sb_work", bufs=1))
        band_dram = ctx.enter_context(
            tc.tile_pool(name="bands", space="DRAM", bufs=1))

        # ---- helpers ----
        def proj_tile(out_sb, rhs_tiles, wn, nt):
            psx = ps.tile([128, 512], F32, name="ps_proj", tag="ps512", bufs=3)
            for kc in range(KC):
                nc.tensor.matmul(
                    out=psx,
                    lhsT=W_t[wn][kc][:, :],
                    rhs=rhs_tiles[kc][:, nt * 512:(nt + 1) * 512],
                    start=(kc == 0), stop=(kc == KC - 1),
                )
            nc.scalar.activation(
                out=out_sb[:, nt * 512:(nt + 1) * 512], in_=psx,
                func=AF.Identity, bias=b_t[wn], scale=1.0,
            )

        cast_flip = [0]

        def band_cast(dst, src):
            # 2:1 DVE:ACT for band psum evacuation
            if cast_flip[0] % 3 != 2:
                nc.vector.tensor_copy(dst, src)
            else:
                nc.scalar.copy(dst, src)
            cast_flip[0] += 1

        def emit_proj_qk(b):
            proj_tile(q2T, xT_t, "q", b)
            proj_tile(k2T, xT_t, "k", b)

        def emit_proj_v(b):
            proj_tile(v2T, xT_t, "v", b)
            for t in range(b * 4, b * 4 + 4):
                pst = ps.tile([128, 128], F16, name="ps_vt", tag="ps_t",
                              bufs=1)
                nc.tensor.transpose(pst, v2T[:, t * 128:(t + 1) * 128], ident)
                nc.vector.tensor_copy(vtok[t][:, 0:64], pst[:, 0:64])
                nc.vector.tensor_copy(vtok[t][:, 65:129], pst[:, 64:128])
                nc.gpsimd.memset(vtok[t][:, 64:65], 1.0)
                nc.gpsimd.memset(vtok[t][:, 129:130], 1.0)

        # --- B1: score bands for one batch: matmul -> sbuf -> dram ---
        c2p_bd = {}         # (b, h) -> dram tile [512, 1024] (pitch-1024 skew)
        p2c_bd = {}         # (b, h) -> dram tile [128, 4*BAND]

        def emit_b1(b):
            csb = {h: sb_band.tile([128, 4 * BAND], F16, name=f"c2p_sb{h}",
                                   tag=f"c2p_sb{h}", bufs=3) for h in range(HC)}
            psb = {h: sb_band.tile([128, 4 * BAND], F16, name=f"p2c_sb{h}",
                                   tag=f"p2c_sb{h}", bufs=3) for h in range(HC)}
            for blk in range(4):
                c0 = 128 * (3 - blk)
                cs = slice(b * 512 + blk * 128, b * 512 + (blk + 1) * 128)
                for src2T, pos, stage in ((q2T, posk, csb), (k2T, posq, psb)):
                    for h in range(HC):
                        hs = slice(h * 64, (h + 1) * 64)
                        pm = ps.tile([128, 512], F32, name="ps_bm",
                                     tag="ps512", bufs=3)
                        nc.tensor.matmul(
                            out=pm, lhsT=src2T[hs, cs],
                            rhs=pos[hs, c0:c0 + 512], start=True, stop=True)
                        pl = ps.tile([128, 128], F32, name="ps_bl",
                                     tag="ps512", bufs=3)
                        nc.tensor.matmul(
                            out=pl, lhsT=src2T[hs, cs],
                            rhs=pos[hs, c0 + 512:c0 + BAND],
                            start=True, stop=True)
                        o = blk * BAND
                        band_cast(stage[h][:, o:o + 512], pm)
                        band_cast(stage[h][:, o + 512:o + BAND], pl)
            for h in range(HC):
                # c2p: strided write into a [512, 1024]-pitch buffer so the
                # skew read is a single 2D AP (flat offset 1023*q + 512 + k)
                bdr = band_dram.tile([512, 1024], F16, name=f"c2pb_{b}{h}",
                                     tag=f"c2p_dram_{b}{h}", bufs=1)
                dst = bass.AP(bdr.tensor, bdr.offset + 384,
                              [[1024, 128], [130944, 4], [1, BAND]])
                nc.sync.dma_start(
                    out=dst, in_=csb[h].rearrange("p (g j) -> p g j", g=4))
                c2p_bd[(b, h)] = bdr
                # p2c: flat [128, 4*BAND]; skew read offset k*2559+kb*640+128+q
                bdr = band_dram.tile([128, 4 * BAND], F16, name=f"p2cb_{b}{h}",
                                     tag=f"p2c_dram_{b}{h}", bufs=1)
                nc.gpsimd.dma_start(out=bdr, in_=psb[h])
                p2c_bd[(b, h)] = bdr

        # --- B2: attention for one batch ---
        # constant exp bias keeps f16 E and the f16-transposed Z in range;
        # it cancels exactly in the final E@v / Z normalization
        def emit_b2(b):
            ostage = sb_work.tile([128, 512], F32, name="ostage",
                                  tag="ostage", bufs=2)
            for h in range(HC):
                hs = slice(h * 64, (h + 1) * 64)
                # merged skew reads: one transpose DMA (c2p) + one
                # accumulating DMA (p2c) build t_sb[k, kb*512 + q]
                t_sb = sb_work.tile([128, 2048], F16, name="t_sb",
                                    tag="t_sb", bufs=3)
                t3 = t_sb.rearrange("p (a j) -> p a j", a=4)
                bdr = c2p_bd[(b, h)]
                nc.sync.dma_start_transpose(
                    out=t3, in_=bass.AP(bdr.tensor, bdr.offset + 512,
                                        [[1023, 512], [1, 512]]))
                bdr = p2c_bd[(b, h)]
                nc.gpsimd.dma_start(
                    out=t3, in_=bass.AP(bdr.tensor, bdr.offset + 128,
                                        [[4 * BAND - 1, 128], [640, 4],
                                         [1, 512]]),
                    accum_op=ALU.add)

                # QK then scores: s = bias + qk (in place), e = exp (in place)
                pqk = []
                for kb in range(4):
                    ks = slice(b * 512 + kb * 128, b * 512 + (kb + 1) * 128)
                    ps_qk = ps.tile([128, 512], F32, name="ps_qk",
                                    tag="ps_qk", bufs=2)
                    nc.tensor.matmul(
                        out=ps_qk, lhsT=k2T[hs, ks],
                        rhs=q2T[hs, b * 512:(b + 1) * 512],
                        start=True, stop=True)
                    pqk.append(ps_qk)
                for kb in range(4):
                    sl = t_sb[:, kb * 512:(kb + 1) * 512]
                    nc.vector.tensor_tensor(out=sl, in0=sl, in1=pqk[kb],
                                            op=ALU.add)
                nc.scalar.activation(out=t_sb, in_=t_sb, func=AF.Exp,
                                     scale=SCALE, bias=exp_bias)

                ps_pv = ps.tile([65, 512], F32, name="ps_pv", tag="ps_pv",
                                bufs=2)
                for kb in range(4):
                    nc.tensor.matmul(
                        out=ps_pv, lhsT=vtok[b * 4 + kb][:, h * 65:h * 65 + 65],
                        rhs=t_sb[:, kb * 512:(kb + 1) * 512],
                        start=(kb == 0), stop=(kb == 3))

                # --- finalize: out^T [65, 512] -> transpose -> /Z -> stage ---
                o2T = sb_work.tile([65, 512], F16, name="o2T", tag="o2T",
                                   bufs=2)
                nc.scalar.copy(o2T, ps_pv)
                for qc in range(4):
                    psT = ps.tile([128, 65], F16, name="psT", tag="ps_t",
                                  bufs=1)
                    nc.tensor.transpose(psT, o2T[:, qc * 128:(qc + 1) * 128],
                                        ident[0:65, 0:65])
                    zrec = sb_work.tile([128, 1], F32, name="zrec",
                                        tag="zrec", bufs=8)
                    nc.vector.reciprocal(zrec, psT[:, 64:65])
                    nc.vector.tensor_scalar_mul(
                        ostage[:, qc * 128 + h * 64:qc * 128 + (h + 1) * 64],
                        psT[:, 0:64], zrec)
            # one merged output write per batch
            dst = bass.AP(out_d.tensor, out_d.offset + b * 65536,
                          [[128, 128], [16384, 4], [1, 128]])
            nc.scalar.dma_start(
                out=dst, in_=ostage.rearrange("p (g j) -> p g j", g=4))

        # ---- emission: 3-deep software pipeline ----
        for nt in range(R // 512):
            proj_tile(posk, relT_t, "k", nt)
            proj_tile(posq, relT_t, "q", nt)
        emit_proj_qk(0)
        emit_b1(0)
        emit_proj_v(0)
        emit_proj_qk(1)
        emit_b1(1)
        emit_proj_v(1)
        emit_proj_qk(2)
        emit_b1(2)
        emit_proj_v(2)
        emit_b2(0)
        emit_proj_qk(3)
        emit_b1(3)
        emit_proj_v(3)
        emit_b2(1)
        emit_b2(2)
        emit_b2(3)


_NC_CACHE = None


def _get_nc():
    global _NC_CACHE
    if _NC_CACHE is None:
        _NC_CACHE = build_nc()
    return _NC_CACHE


def make_in_maps(inputs):
    x = np.asarray(inputs["x"], np.float32)
    rel = np.asarray(inputs["rel_embeddings"], np.float32)
    Wq = np.asarray(inputs["Wq"], np.float32)
    Wk = np.asarray(inputs["Wk"], np.float32)
    Wv = np.asarray(inputs["Wv"], np.float32)
    bq = np.asarray(inputs["bq"], np.float32)
    bk = np.asarray(inputs["bk"], np.float32)
    bv = np.asarray(inputs["bv"], np.float32)

    xT = np.ascontiguousarray(x.reshape(T, DIM).T).astype(np.float16)
    relT = np.ascontiguousarray(rel[::-1].T).astype(np.float16)
    in_maps = []
    for c in range(NCORES):
        sl = slice(c * 128, (c + 1) * 128)
        in_maps.append({
            "xT": xT,
            "relT": relT,
            "Wq": np.ascontiguousarray(Wq[:, sl]).astype(np.float16),
            "Wk": np.ascontiguousarray(Wk[:, sl]).astype(np.float16),
            "Wv": np.ascontiguousarray(Wv[:, sl]).astype(np.float16),
            "bq": np.ascontiguousarray(bq[sl]).reshape(128, 1),
            "bk": np.ascontiguousarray(bk[sl]).reshape(128, 1),
            "bv": np.ascontiguousarray(bv[sl]).reshape(128, 1),
        })
    return in_maps


def kernel(**inputs):
    nc = _get_nc()
    in_maps = make_in_maps(inputs)
    res = run_bass_kernel_spmd(nc, in_maps, list(range(NCORES))).results
    out = np.concatenate([res[c]["out"] for c in range(NCORES)], axis=1)
    return out.reshape(B, S, DIM).astype(np.float32)
